# revision 1
# baseline (speedup 1.0000x reference)
"""Trainium2 Bass kernel: conv3d(16,3x3x3,VALID) -> channel softmax -> 2x maxpool3d(2) twice.

Full inputs: x [8,3,96,96,96] f32, w [16,3,3,3,3] f32, b [16] f32.
Output: [8,16,23,23,23] f32.

Sharding: data-parallel over batch N=8 across 8 NeuronCores (1 sample/core).

Per-core algorithm (sample x_i [3,96,96,96] -> out_i [16,23,23,23]):
  Only conv outputs d,h,w in [0,92) survive the two maxpools (23*4=92), so we
  compute conv on a 92^3 grid, grouped as 23 d-quads x 23 h-quads x 92 w.

  Layout trick: one PSUM tile [128, 368] holds 8 h-quads x 16 channels on the
  partition axis (partition p = 16*g + c, g = h-quad index within a chunk of 8)
  and (h_local 4, w 92) on the free axis, for one conv-output depth d.
  The conv is a matmul with a block-diagonal stationary operand:
    lhsT [108, 64] = diag blocks of w[(cin,kd,kh), cout] per kw tap (K=27*4),
  accumulating 3 kw taps into PSUM via column-shifted views of one rhs tile.
  Two concurrent matmuls (tile_position col groups (0,0) and (0,64)) fill all
  128 partitions.

  Softmax+pool in log domain: y = (x+b) - ln(sum_c exp(x_c+b)) and
  maxpool(softmax) = exp(maxpool(y)) since exp is monotone. So:
    exp:  ACT e = exp(logits + b) -> SBUF as float32r (12-bit mantissa)
    sum:  PE  S = lambda * sum_c e, group-BROADCAST to all 128 partitions
          via a [128,128] block-ones lhsT; lambda = 2^-38 keeps ln's input
          inside the ACT Ln LUT's valid range [e^-44.5, e^44.5]
    ln:   ACT full-f32 ln(S) (f32r would round |ln S|~40 too coarsely)
    sub:  DVE y = logits - ln(S)
    pool: DVE reduce_max over w, then h_local, then d (all free-axis APs)
    out:  ACT exp(y_pool + b - 38*ln2), one 3-dim DMA per (dq, chunk).

  Conv matmuls are fp16 hi/lo 3-term (xh*wh + xh*wl + xl*wh, ~2e-5 rel):
  fp32 matmul is 4 cycles/row and float32r matmuls may only write PSUM
  partition 0 (ISA s3d3 check), which would kill the col-group concurrency.
  The sum matmul stays float32r at base partition 0 (1 cycle/row, N>=256).

  DMA: each dma_start costs ~0.76us fixed regardless of size, and DMA APs
  are limited to 3 dims with a contiguous last dim. So x is first staged
  into B[27 taps, d, h, w] in DRAM (54 big HBM->HBM copies, both HWDGE
  rings), after which each rhs im2col tile quarter loads as a single
  27-partition DMA (288 loads total instead of 2592 3-partition pieces).
"""

import numpy as np
from contextlib import ExitStack

import concourse.bass as bass
import concourse.bacc as bacc
import concourse.tile as tile
from concourse import mybir
from concourse.bass_utils import run_bass_kernel_spmd

F32 = mybir.dt.float32
F32R = mybir.dt.float32r
BF16 = mybir.dt.bfloat16
F16 = mybir.dt.float16

N_CORES = 8
CIN, S = 3, 96
COUT = 16
Q = 23          # pooled output size per dim
DU = 92         # conv positions used per dim (23*4)
NW = 94         # w extent loaded (92 + 2 halo for kw shifts)

CONV_MODE = "hilo"   # "f32r" | "hilo" (fp16 hi/lo 3-term)

_cache: dict = {}


def _emit(nc, xs, wls, ws_, wbc_, bias_, bias2_, out_, mode, chunks=(0, 8, 15),
          dq0s=None, ndq_cap=4, repeat=1, stage="full"):
    """Build the Tile kernel. xs: list of x dram APs (1 for f32r, 2 for bf16x2
    [hi, lo]); wls: list of lhsT dram APs ([108,3,64] each)."""
    S2 = S * S          # 9216
    S3 = S * S * S      # 884736
    if dq0s is None:
        dq0s = range(0, Q, 4)

    with tile.TileContext(nc) as tc, ExitStack() as ctx:
        consts = ctx.enter_context(tc.tile_pool(name="consts", bufs=1))
        rhsp = ctx.enter_context(tc.tile_pool(name="rhs", bufs=3))
        ep = ctx.enter_context(tc.tile_pool(name="e", bufs=3))
        ellp = ctx.enter_context(tc.tile_pool(name="ell", bufs=3))
        yp = ctx.enter_context(tc.tile_pool(name="y", bufs=3))
        wpp = ctx.enter_context(tc.tile_pool(name="wp", bufs=2))
        hpp = ctx.enter_context(tc.tile_pool(name="hp", bufs=2))
        finp = ctx.enter_context(tc.tile_pool(name="fin", bufs=2))
        outp = ctx.enter_context(tc.tile_pool(name="outt", bufs=2))
        psl = ctx.enter_context(tc.tile_pool(name="psl", bufs=3, space="PSUM"))
        pss = ctx.enter_context(tc.tile_pool(name="pss", bufs=1, space="PSUM"))

        rhs_dt = F32R if mode == "f32r" else F16
        
        # constants
        wlts = []
        for i, wl in enumerate(wls):
            t = consts.tile([108, 3, 64], rhs_dt, tag=f"wl{i}")
            nc.sync.dma_start(out=t, in_=wl[:])
            wlts.append(t)
        wst = consts.tile([128, 128], F32R, tag="ws")
        nc.sync.dma_start(out=wst, in_=ws_[:])
        biast = consts.tile([128, 1], F32, tag="bias")
        nc.sync.dma_start(out=biast, in_=bias_[:])
        biast2 = consts.tile([128, 1], F32, tag="bias2")
        nc.sync.dma_start(out=biast2, in_=bias2_[:])

        nx = len(xs)  # matmul terms per tap (1 or 3 -> hi/lo operand pairs)
        # term -> (x operand index, lhsT operand index)
        if mode == "f32r":
            terms = [(0, 0)]
        else:
            terms = [(0, 0), (0, 1), (1, 0)]  # xh*wh + xh*wl + xl*wh

        # --- staging: B[v][slot=(ci,kd,kh), d, h, w] = x[ci, d+kd, h+kh, w] ---
        # One HBM->HBM copy per slot (27 per operand). After staging, each
        # rhs tile quarter loads with a single 27-partition DMA (the 3-dim
        # DMA AP limit makes direct strided loads need 3-partition pieces,
        # and each dma_start costs ~0.76us fixed).
        DH, HH, HW = 94, 93, 96
        dramp = ctx.enter_context(tc.tile_pool(name="dram", bufs=1, space="DRAM"))
        Bs = []
        for v in range(nx):
            bt = dramp.tile([27, DH, HH, HW], rhs_dt, tag=f"B{v}")
            for ci in range(CIN):
                for kd in range(3):
                    for kh in range(3):
                        slot = 9 * ci + 3 * kd + kh
                        src = bass.AP(
                            tensor=xs[v],
                            offset=ci * S3 + kd * S2 + kh * S,
                            ap=[[S2, DH], [1, HH * HW]],
                        )
                        eng = nc.scalar if (slot % 2) else nc.sync
                        eng.dma_start(
                            out=bt[slot].rearrange("d h w -> d (h w)"), in_=src)
            Bs.append(bt)

        for _rep in range(repeat):
          for hq0 in chunks:
            for dq0 in dq0s:
                ndq = min(ndq_cap, Q - dq0)
                E = 4 * ndq  # depths staged in this rhs tile group (16 or 12)
                # --- load rhs im2col tiles [108, E, 4*96] for 4 d-quads ---
                # rhs[v][a]: operand v (hi/lo), half a (h-quad groups 4a..4a+3)
                # partition r = 27*g4 + 9*ci + 3*kd + kh; free = (d, h_local*96+w)
                EL = 1 if stage == "dmat" else E
                rhs = [[None, None] for _ in range(nx)]
                for v in range(nx):
                    for a in (0, 1):
                        t = rhsp.tile([108, 16, 4 * S], rhs_dt, tag=f"rhs{v}{a}")
                        rhs[v][a] = t
                        for g4 in range(4):
                            hq = hq0 + 4 * a + g4
                            src = bass.AP(
                                tensor=Bs[v].tensor,
                                offset=(Bs[v].offset
                                        + (4 * dq0) * HH * HW + (4 * hq) * HW),
                                ap=[[DH * HH * HW, 27], [HH * HW, EL], [1, 4 * S]],
                            )
                            eng = nc.scalar if (g4 % 2) else nc.sync
                            eng.dma_start(
                                out=t[27 * g4:27 * g4 + 27, 0:EL, :], in_=src)

                for dq in range(dq0, dq0 + ndq):
                    dsi0 = 4 * (dq - dq0)
                    if stage in ("dmao", "dmat"):
                        continue
                    hp = hpp.tile([128, 4, Q], F32)
                    if stage == "dma":
                        nc.vector.memset(hp, 0.0)
                    for pr in ((0, 1) if stage not in ("dma", "dmao") else ()):
                        logits = psl.tile([128, 2, 512], F32)
                        first = {(a, dl): True for a in (0, 1) for dl in (0, 1)}
                        nmm = 3 * len(terms)
                        cnt = {(a, dl): 0 for a in (0, 1) for dl in (0, 1)}
                        for dl in (0, 1):
                            dsi = dsi0 + 2 * pr + dl
                            for kw in range(3):
                                for a in (0, 1):
                                    for (xi, wi) in terms:
                                        lhsT = wlts[wi][:, kw, :]
                                        r = rhs[xi][a][:, dsi, :].rearrange(
                                            "p (hl w) -> p hl w", hl=4,
                                        )[:, :, kw:kw + DU]
                                        cnt[(a, dl)] += 1
                                        nc.tensor.matmul(
                                            out=logits[64 * a:64 * a + 64, dl, 0:368],
                                            lhsT=lhsT,
                                            rhs=r,
                                            start=first[(a, dl)],
                                            stop=(cnt[(a, dl)] == nmm),
                                            skip_group_check=True,
                                        )
                                        first[(a, dl)] = False
                        if stage == "conv":
                            wp0 = wpp.tile([128, 2, 4, Q], F32)
                            nc.vector.reduce_max(
                                out=wp0,
                                in_=logits[:, :, 0:368].rearrange(
                                    "p d (hl wq wl) -> p d hl wq wl",
                                    hl=4, wq=Q),
                                axis=mybir.AxisListType.X,
                            )
                            nc.vector.reduce_max(
                                out=hp[:, 2 * pr:2 * pr + 2, :],
                                in_=wp0.rearrange("p d hl wq -> p d wq hl"),
                                axis=mybir.AxisListType.X,
                            )
                            continue
                        # exp(logits + b) for both d of the pair, PSUM -> SBUF
                        e = ep.tile([128, 2, 368], F32R)
                        nc.scalar.activation(
                            out=e, in_=logits[:, :, 0:368],
                            func=mybir.ActivationFunctionType.Exp,
                            bias=biast[:, 0:1],
                        )
                        # per-group channel sums, broadcast to all 128
                        # partitions in one matmul: lhsT[k, p] = (k//16==p//16)
                        s = pss.tile([128, 2, 512], F32)
                        for dl in (0, 1):
                            nc.tensor.matmul(
                                out=s[:, dl, 0:368],
                                lhsT=wst,
                                rhs=e[:, dl, :],
                                start=True, stop=True,
                            )
                        # ln(s) in full fp32 (f32r would round |ln s|~40 too
                        # coarsely), then y = logits - ln(s) on DVE
                        ell = ellp.tile([128, 2, 368], F32)
                        nc.scalar.activation(
                            out=ell, in_=s[:, :, 0:368],
                            func=mybir.ActivationFunctionType.Ln,
                        )
                        y = yp.tile([128, 2, 368], F32)
                        nc.vector.tensor_tensor(
                            out=y, in0=logits[:, :, 0:368], in1=ell,
                            op=mybir.AluOpType.subtract,
                        )
                        # w-pool: [128, 2, 4, 23, 4] -> [128, 2, 4, 23]
                        wp = wpp.tile([128, 2, 4, Q], F32)
                        nc.vector.reduce_max(
                            out=wp,
                            in_=y.rearrange(
                                "p d (hl wq wl) -> p d hl wq wl", hl=4, wq=Q),
                            axis=mybir.AxisListType.X,
                        )
                        # h-pool: reduce over h_local -> hp[:, 2*pr:2*pr+2, :]
                        nc.vector.reduce_max(
                            out=hp[:, 2 * pr:2 * pr + 2, :],
                            in_=wp.rearrange("p d hl wq -> p d wq hl"),
                            axis=mybir.AxisListType.X,
                        )
                    # d-pool over the quad
                    fin = finp.tile([128, Q], F32)
                    nc.vector.reduce_max(
                        out=fin,
                        in_=hp.rearrange("p d wq -> p wq d"),
                        axis=mybir.AxisListType.X,
                    )
                    # back to probability domain, + bias inside exp
                    ot = outp.tile([128, Q], F32)
                    nc.scalar.activation(
                        out=ot, in_=fin,
                        func=mybir.ActivationFunctionType.Exp,
                        bias=biast2[:, 0:1],
                    )
                    if stage == "dmao":
                        continue
                    # SBUF side stays a plain [128, Q] AP (partition-major
                    # order is already g-major); the DRAM side carries the
                    # (g, c, w) pattern. Split-partition SBUF APs mislower.
                    dma_out_eng = nc.sync
                    dma_out_eng.dma_start(
                        out=out_[:][:, dq, hq0:hq0 + 8, :].rearrange(
                            "c g w -> g c w"),
                        in_=ot,
                    )


def _build(mode, chunks=(0, 8, 15), dq0s=None, ndq_cap=4, repeat=1, stage="full"):
    nc = bacc.Bacc(name="conv_softmax_pool")
    if mode == "f32r":
        xs = [nc.declare_dram_parameter("x", [CIN, S, S, S], F32R, isOutput=False)]
        wls = [nc.declare_dram_parameter("wl0", [108, 3, 64], F32R, isOutput=False)]
    else:
        xs = [
            nc.declare_dram_parameter("xh", [CIN, S, S, S], F16, isOutput=False),
            nc.declare_dram_parameter("xl", [CIN, S, S, S], F16, isOutput=False),
        ]
        wls = [
            nc.declare_dram_parameter("wl0", [108, 3, 64], F16, isOutput=False),
            nc.declare_dram_parameter("wl1", [108, 3, 64], F16, isOutput=False),
        ]
    ws_ = nc.declare_dram_parameter("ws", [128, 128], F32R, isOutput=False)
    wbc_ = None
    bias_ = nc.declare_dram_parameter("bias", [128, 1], F32, isOutput=False)
    bias2_ = nc.declare_dram_parameter("bias2", [128, 1], F32, isOutput=False)
    out_ = nc.declare_dram_parameter("out", [COUT, Q, Q, Q], F32, isOutput=True)
    _emit(nc, xs, wls, ws_, wbc_, bias_, bias2_, out_, mode, chunks=chunks,
          dq0s=dq0s, ndq_cap=ndq_cap, repeat=repeat, stage=stage)
    nc.finalize()
    return nc


def _host_prep(w, b, mode):
    """Build lhsT block-diagonal matrices and softmax helper matrices."""
    # wl[r, kw, m]: r = 27g + 9ci + 3kd + kh, m = 16g + c  (g = 0..3)
    def blockdiag(wm):  # wm [cout, cin, kd, kh, kw] float
        wl = np.zeros((108, 3, 64), np.float32)
        for g in range(4):
            for ci in range(CIN):
                for kd in range(3):
                    for kh in range(3):
                        wl[27 * g + 9 * ci + 3 * kd + kh, :, 16 * g:16 * g + 16] = \
                            wm[:, ci, kd, kh, :].T
        return wl

    # 2^-38 scale keeps ln(lambda*s) inside the ACT Ln LUT's valid input
    # range [e^-44.5, e^44.5]; compensated in the final exp bias.
    ws_ = np.zeros((128, 128), np.float32)
    for g in range(8):
        ws_[16 * g:16 * g + 16, 16 * g:16 * g + 16] = 2.0 ** -38
    wbc_ = None
    bias_ = np.tile(b.astype(np.float32), 8).reshape(128, 1)
    # y = logit - ln(lambda*s) = logit - ln s + 38ln2, so the final
    # exp needs bias2 = b - 38ln2 to recover exp(logit + b - ln s).
    bias2_ = bias_ - np.float32(38.0 * np.log(2.0))

    if mode == "f32r":
        wls = [blockdiag(w.astype(np.float32))]
    else:
        wh = w.astype(np.float32).astype(np.float16)
        wlo = (w.astype(np.float32) - wh.astype(np.float32)).astype(np.float16)
        wls = [blockdiag(wh.astype(np.float32)).astype(np.float16),
               blockdiag(wlo.astype(np.float32)).astype(np.float16)]
    return wls, ws_, wbc_, bias_, bias2_


def kernel(x, w, b):
    mode = CONV_MODE
    key = ("nc", mode)
    if key not in _cache:
        _cache[key] = _build(mode)
    nc = _cache[key]

    x = np.asarray(x, np.float32)
    w = np.asarray(w, np.float32)
    b = np.asarray(b, np.float32)
    wls, ws_, wbc_, bias_, bias2_ = _host_prep(w, b, mode)

    in_maps = []
    for i in range(N_CORES):
        m = {"ws": ws_, "bias": bias_, "bias2": bias2_}
        if mode == "f32r":
            m["x"] = np.ascontiguousarray(x[i])
            m["wl0"] = wls[0]
        else:
            xh = x[i].astype(np.float16)
            xl = (x[i] - xh.astype(np.float32)).astype(np.float16)
            m["xh"] = np.ascontiguousarray(xh)
            m["xl"] = np.ascontiguousarray(xl)
            m["wl0"] = wls[0]
            m["wl1"] = wls[1]
        in_maps.append(m)

    res = run_bass_kernel_spmd(nc, in_maps, core_ids=list(range(N_CORES)))
    return np.stack([r["out"] for r in res.results]).astype(np.float32)

